# revision 1
# baseline (speedup 1.0000x reference)
"""Trainium2 Bass kernel for DWConvBlock3D:
depthwise 3x3x3 conv (pad 1) + InstanceNorm3d + ReLU on x:(2,64,64,128,128) f32.

Strategy (8 NeuronCores, channel sharding => zero communication):
  - Each core owns 8 channels x 2 batches = 16 (b,c) "pairs".
  - Layout per pair: H=128 on SBUF partitions, (D,W) on the free dim.
  - The conv runs on TensorE in fp16: for each (kd,kw) of the 9 off-H taps, a
    128x128 banded matrix (3 diagonals = the kh taps, built host-side from w)
    multiplies a (d,w)-shifted view of the x tile; the 9 matmuls accumulate in
    fp32 PSUM.  H zero-padding falls out of the band structure; D/W edges are
    handled by clipping the shifted matmuls (PSUM has_written gives correct
    first-write-overwrite semantics; the center tap goes first with start=True
    so the whole bank is initialized).
  - InstanceNorm stats (fp32): sum(y) comes free from the PSUM->SBUF eviction
    (ScalarE activation-copy accum_out); sum(y^2) from one VectorE
    scalar_tensor_tensor pass; cross-partition reduction via GpSimd
    partition_all_reduce (keeps TensorE's queue free of tiny matmuls).
  - Final (y*scale+bias, ReLU) is a single in-place ScalarE activation with
    per-partition scale/bias columns (already replicated by the all-reduce).
"""

import sys

if "/opt/trn_rl_repo" not in sys.path:
    sys.path.insert(0, "/opt/trn_rl_repo")

import numpy as np

B, C, D, H, W = 2, 64, 64, 128, 128
N_CORES = 8
CH_PER_CORE = C // N_CORES  # 8
N_PAIRS = B * CH_PER_CORE  # 16
WP = W + 2  # host-padded W (zero borders) -> contiguous DMA, free w-shifts
FREE = D * W  # 8192 free elements per partition per pair
NV = D * H * W  # normalization element count per (b,c)
EPS = 1e-5
CHUNK_D = 4  # d-slices per PSUM bank chunk (4*128 = 512 fp32 = 1 bank)
# center tap first: start=True covers the full bank (edge taps are clipped)
TAP_ORDER = [(1, 1), (0, 0), (0, 1), (0, 2), (1, 0), (1, 2), (2, 0), (2, 1), (2, 2)]


def build_program(d=D, n_pairs=N_PAIRS, ch_per_core=CH_PER_CORE):
    import concourse.bacc as bacc
    import concourse.mybir as mybir
    from concourse import bass_isa
    from concourse.tile import TileContext

    free = d * W
    nv = d * H * W
    n_chunks = d // CHUNK_D
    groups = []
    left = n_chunks
    while left > 0:
        g = min(8, left)
        groups.append(g)
        left -= g

    f32 = mybir.dt.float32
    f16 = mybir.dt.float16
    nc = bacc.Bacc("TRN2", target_bir_lowering=False, debug=False, num_devices=N_CORES)

    xs = nc.dram_tensor("xs", [n_pairs, H, d, WP], f16, kind="ExternalInput")
    bands = nc.dram_tensor(
        "bands", [H, ch_per_core, 3, 3, H], f16, kind="ExternalInput"
    )  # [h_in, ci, kd, kw, h_out]
    gb = nc.dram_tensor("gb", [128, 2 * n_pairs], f32, kind="ExternalInput")
    out = nc.dram_tensor("out", [n_pairs, H, free], f32, kind="ExternalOutput")

    with TileContext(nc) as tc:
        with (
            tc.tile_pool(name="singles", bufs=1) as singles,
            tc.tile_pool(name="xp", bufs=3) as xpool,
            tc.tile_pool(name="yp", bufs=3) as ypool,
            tc.tile_pool(name="st", bufs=3) as stats,
            tc.tile_pool(name="psmm", bufs=8, space="PSUM") as psum_mm,
        ):
            band_sb = singles.tile([H, ch_per_core, 3, 3, H], f16)
            gb_sb = singles.tile([128, 2 * n_pairs], f32)
            nc.sync.dma_start(out=gb_sb[:], in_=gb[:])

            for p in range(n_pairs):
                ci = p % ch_per_core

                xt = xpool.tile([H, d, WP], f16, tag="xt")
                nc.sync.dma_start(out=xt[:], in_=xs[p])
                if p < ch_per_core:
                    # just-in-time per-channel band load (keeps startup short)
                    nc.sync.dma_start(out=band_sb[:, ci], in_=bands[:, ci])

                y = ypool.tile([H, free], f32, tag="y")
                sums = stats.tile([128, n_chunks], f32, tag="sums")
                st2 = stats.tile([128, 2], f32, tag="st2")

                # ---- depthwise conv: 9 banded matmuls per chunk, PSUM-accumulated
                chunk0 = 0
                for gsize in groups:
                    chunks = range(chunk0, chunk0 + gsize)
                    chunk0 += gsize
                    ps = {
                        c: psum_mm.tile(
                            [128, CHUNK_D, W], f32, tag="mm", name=f"mm_{p}_{c}"
                        )
                        for c in chunks
                    }
                    for t9, (kd, kw) in enumerate(TAP_ORDER):
                        lhsT = band_sb[:, ci, kd, kw, :]
                        for c in chunks:
                            d0 = c * CHUNK_D
                            lo_d = max(0, d0 + kd - 1)
                            hi_d = min(d, d0 + CHUNK_D + kd - 1)
                            od = lo_d - (d0 + kd - 1)
                            nd = hi_d - lo_d
                            nc.tensor.matmul(
                                ps[c][:, od : od + nd, :],
                                lhsT,
                                xt[:, lo_d:hi_d, kw : kw + W],
                                start=(t9 == 0),
                                stop=(t9 == 8),
                                skip_group_check=True,
                            )
                    # ---- evict PSUM -> y (ScalarE); per-chunk sum(y) via accum_out
                    for c in chunks:
                        nc.scalar.activation(
                            out=y[:, c * CHUNK_D * W : (c + 1) * CHUNK_D * W],
                            in_=ps[c][:],
                            func=mybir.ActivationFunctionType.Copy,
                            accum_out=sums[:, c : c + 1],
                        )

                # ---- per-partition stats
                nc.vector.tensor_reduce(
                    out=st2[:, 0:1], in_=sums[:], axis=mybir.AxisListType.X,
                    op=mybir.AluOpType.add,
                )
                # sum(y^2) in one DVE pass; squares go to the dead x tile
                sq_scratch = xt[:].rearrange("p a b -> p (a b)")[:, 0:free]
                nc.vector.scalar_tensor_tensor(
                    out=sq_scratch, in0=y[:], scalar=1.0, in1=y[:],
                    op0=mybir.AluOpType.mult, op1=mybir.AluOpType.mult,
                    accum_out=st2[:, 1:2],
                )

                # ---- all-reduce across partitions (GpSimd) -> every partition
                # holds (sum, sumsq); the stats math then runs replicated
                ast = stats.tile([128, 2], f32, tag="ast")
                nc.gpsimd.partition_all_reduce(
                    ast[:], st2[:], 128, bass_isa.ReduceOp.add
                )

                sm = stats.tile([128, 10], f32, tag="sm")
                mean, ex2 = sm[:, 0:1], sm[:, 1:2]
                msq, vpe = sm[:, 2:3], sm[:, 3:4]
                std, r0 = sm[:, 4:5], sm[:, 5:6]
                t1, t2 = sm[:, 6:7], sm[:, 7:8]
                t4, rr = sm[:, 8:9], sm[:, 9:10]
                nc.vector.tensor_scalar_mul(mean, ast[:, 0:1], 1.0 / nv)
                nc.vector.tensor_scalar_mul(ex2, ast[:, 1:2], 1.0 / nv)
                nc.vector.tensor_mul(msq, mean, mean)
                nc.vector.tensor_sub(vpe, ex2, msq)
                nc.vector.tensor_scalar_add(vpe, vpe, EPS)
                nc.scalar.activation(std, vpe, mybir.ActivationFunctionType.Sqrt)
                nc.vector.reciprocal(r0, std)
                # one Newton step: r = r0*(1.5 - 0.5*vpe*r0^2)
                nc.vector.tensor_mul(t1, r0, r0)
                nc.vector.tensor_mul(t2, t1, vpe)
                nc.vector.tensor_scalar(
                    t4, t2, -0.5, 1.5, op0=mybir.AluOpType.mult, op1=mybir.AluOpType.add
                )
                nc.vector.tensor_mul(rr, r0, t4)

                sb2 = stats.tile([128, 2], f32, tag="sb2")
                sc, bi = sb2[:, 0:1], sb2[:, 1:2]
                # scale = gamma * rstd ; bias = beta - mean*scale
                nc.vector.tensor_mul(sc, rr, gb_sb[:, p : p + 1])
                nc.vector.tensor_mul(t1, mean, sc)
                nc.vector.tensor_sub(bi, gb_sb[:, n_pairs + p : n_pairs + p + 1], t1)

                # ---- fused normalize + ReLU (in place), then store.
                # split halves so the ScalarE apply overlaps the out-DMA
                hf = free // 2
                for h2 in range(2):
                    ysl = y[:, h2 * hf : (h2 + 1) * hf]
                    nc.scalar.activation(
                        out=ysl,
                        in_=ysl,
                        func=mybir.ActivationFunctionType.Relu,
                        scale=sc,
                        bias=bi,
                    )
                    nc.gpsimd.dma_start(
                        out=out[p][:, h2 * hf : (h2 + 1) * hf], in_=ysl
                    )

    nc.compile()
    return nc


_NC_CACHE = None


def _get_program():
    global _NC_CACHE
    if _NC_CACHE is None:
        _NC_CACHE = build_program()
    return _NC_CACHE


def make_core_inputs(x, w, gamma, beta, core):
    cs = slice(CH_PER_CORE * core, CH_PER_CORE * (core + 1))
    # (b, ci, d, h, w) -> (b, ci, h, d, w) -> (pair, h, d, w), pair = b*8+ci
    xc = np.zeros((N_PAIRS, H, D, WP), np.float16)
    xc[:, :, :, 1 : W + 1] = (
        np.ascontiguousarray(x[:, cs].transpose(0, 1, 3, 2, 4))
        .reshape(N_PAIRS, H, D, W)
        .astype(np.float16)
    )
    bands = np.zeros((H, CH_PER_CORE, 3, 3, H), np.float32)
    eye0 = np.eye(H, dtype=np.float32)
    eyep = np.eye(H, k=1, dtype=np.float32)  # B[h-1, h]: kh=0 tap
    eyem = np.eye(H, k=-1, dtype=np.float32)  # B[h+1, h]: kh=2 tap
    for ci in range(CH_PER_CORE):
        c = CH_PER_CORE * core + ci
        for kd in range(3):
            for kw in range(3):
                wk = w[c, 0, kd, :, kw]
                bands[:, ci, kd, kw, :] = wk[0] * eyep + wk[1] * eye0 + wk[2] * eyem
    gbv = np.broadcast_to(
        np.concatenate([np.tile(gamma[cs], B), np.tile(beta[cs], B)])
        .astype(np.float32)
        .reshape(1, 2 * N_PAIRS),
        (128, 2 * N_PAIRS),
    ).copy()
    return {"xs": xc, "bands": bands.astype(np.float16), "gb": gbv}


def kernel(x, w, gamma, beta):
    from concourse.bass_utils import run_bass_kernel_spmd

    x = np.asarray(x, dtype=np.float32)
    w = np.asarray(w, dtype=np.float32)
    gamma = np.asarray(gamma, dtype=np.float32)
    beta = np.asarray(beta, dtype=np.float32)

    nc = _get_program()
    in_maps = [make_core_inputs(x, w, gamma, beta, k) for k in range(N_CORES)]
    res = run_bass_kernel_spmd(nc, in_maps, core_ids=list(range(N_CORES)))

    out = np.empty((B, C, D, H, W), np.float32)
    for k in range(N_CORES):
        cs = slice(CH_PER_CORE * k, CH_PER_CORE * (k + 1))
        yc = res.results[k]["out"].reshape(B, CH_PER_CORE, H, D, W)
        out[:, cs] = yc.transpose(0, 1, 3, 2, 4)
    return out

